# revision 8
# baseline (speedup 1.0000x reference)
# MoE kernel for Trainium2 (8 NeuronCores, dff-sharded expert MLP).
#
# Strategy:
#  - Host: gate logits = x @ gate_w, top-2 + softmax, gather tokens per expert
#    into one expert-sorted pair stream of exactly N*TOP_K = 8192 tokens.
#  - Device (core s = dff slice s): every core processes ALL 8192 routed
#    pairs over a 512-wide slice of d_ff: h = gelu(x @ w1[:, slice]);
#    y_partial = h @ w2[slice, :]. Perfectly load-balanced regardless of
#    routing (the expert-parallel alternative pads every core to the max
#    expert load). Token stream is cut into fixed chunks; expert boundaries
#    fall inside chunks as column sub-ranges of the same PSUM tile.
#  - Host: sum the 8 bf16 partials, add b2, scatter-add wts * y back.
import math
from contextlib import ExitStack

import ml_dtypes
import numpy as np

import concourse.bass as bass
import concourse.mybir as mybir
import concourse.tile as tile
from concourse.bass_utils import run_bass_kernel_spmd

D = 1024
DFF = 4096
E = 8
TOP_K = 2
P = 128
KD = D // P        # 8 contraction tiles for GEMM1
S_LOC = DFF // 8   # 512 dff columns per core
NFL = S_LOC // P   # 4 local dff tiles (GEMM1 out / GEMM2 contraction)
ND = D // P        # 8 GEMM2 out tiles
NPAIR = 4096 * TOP_K
CH_PLAN = [128, 384] + [512] * 14 + [384, 128]
assert sum(CH_PLAN) == NPAIR
NCH = len(CH_PLAN)
CH_OFF = [0]
for _s in CH_PLAN:
    CH_OFF.append(CH_OFF[-1] + _s)

BF16 = mybir.dt.bfloat16
F32 = mybir.dt.float32
NP_BF16 = np.dtype(ml_dtypes.bfloat16)

_neff_cache = {}


def _split_multiwait_json(bir_bytes: bytes) -> bytes:
    """The walrus build in this container rejects instructions carrying more
    than one sync wait (or update). Split extras onto adjacent single-wait
    EventSemaphore carriers on the same engine: program order on the engine
    preserves the semantics exactly."""
    import json as _json

    bir = _json.loads(bir_bytes)
    for fn in bir["functions"]:
        for blk in fn["blocks"]:
            insts = blk.get("instructions", [])
            out = []
            for inst in insts:
                si = inst.get("sync_info")
                if si:
                    waits = si.get("on_wait") or []
                    if len(waits) > 1:
                        for i, w in enumerate(waits[:-1]):
                            out.append({
                                "debug": inst.get("debug", 0),
                                "engine": inst["engine"],
                                "ins": [],
                                "name": f"{inst['name']}_w{i}",
                                "opcode": "EventSemaphore",
                                "outs": [],
                                "sync_info": {"on_update": [], "on_wait": [w]},
                            })
                        si["on_wait"] = [waits[-1]]
                out.append(inst)
                if si:
                    ups = si.get("on_update") or []
                    if len(ups) > 1:
                        for i, u in enumerate(ups[1:]):
                            out.append({
                                "debug": inst.get("debug", 0),
                                "engine": inst["engine"],
                                "ins": [],
                                "name": f"{inst['name']}_u{i}",
                                "opcode": "EventSemaphore",
                                "outs": [],
                                "sync_info": {"on_update": [u], "on_wait": []},
                            })
                        si["on_update"] = [ups[0]]
            blk["instructions"] = out
    return _json.dumps(bir).encode()


def _patch_to_json(nc: bass.Bass) -> bass.Bass:
    orig = nc.to_json_bytes
    nc.to_json_bytes = lambda: _split_multiwait_json(orig())
    return nc


def _segments(cnts):
    """Per chunk: list of (expert, col0, col1) in chunk-local columns."""
    st = [0]
    for c in cnts:
        st.append(st[-1] + c)
    segs = []
    for ci in range(NCH):
        o0, o1 = CH_OFF[ci], CH_OFF[ci + 1]
        lst = []
        for e in range(E):
            a, b = max(o0, st[e]), min(o1, st[e + 1])
            if a < b:
                lst.append((e, a - o0, b - o0))
        segs.append(lst)
    return segs


def _build_bass(cnts) -> bass.Bass:
    """One dff-slice of the MoE MLP; identical program on all 8 cores.

    DRAM layouts (rows padded to fixed 8KB width; host packs accordingly):
      xs : [NCH*P, KD*512] bf16; rows c*P+p hold [kd, csz] = xg[o0+col, kd*P+p]
      w1 : [E*P, KD*512] bf16; rows e*P+p hold [kd, f] = w1[e][kd*P+p, slice f]
      w2 : [E*P, NFL*1024] bf16; rows e*P+p hold [fi, dm] = w2[e][slice fi*P+p, dm]
      b1 : [P, E*NFL] f32; [p, e*NFL+fi] = b1[e][slice fi*P+p]
      y  : [NCH*P, ND*512] bf16; rows c*P+p hold [dd, csz] = y_part[o0+col, dd*P+p]
    """
    nc = bass.Bass()
    segs = _segments(cnts)
    xs_h = nc.dram_tensor("xs", [NCH * P, KD * 512], BF16, kind="ExternalInput")
    w1_h = nc.dram_tensor("w1", [E * P, KD * 512], BF16, kind="ExternalInput")
    w2_h = nc.dram_tensor("w2", [E * P, NFL * 1024], BF16, kind="ExternalInput")
    b1_h = nc.dram_tensor("b1", [P, E * NFL], F32, kind="ExternalInput")
    y_h = nc.dram_tensor("y", [NCH * P, ND * 512], BF16, kind="ExternalOutput")

    # first chunk index where each expert appears -> weight prefetch schedule
    first_chunk = {}
    for ci in range(NCH):
        for (e, _, _) in segs[ci]:
            if e not in first_chunk:
                first_chunk[e] = ci
    due = {ci: [] for ci in range(NCH)}
    for e in range(E):
        if e == 0 or e not in first_chunk:
            continue
        due[max(0, first_chunk[e] - 3)].append(e)

    with ExitStack() as ctx:
        tc = ctx.enter_context(tile.TileContext(nc))
        wpool = ctx.enter_context(tc.tile_pool(name="w", bufs=1))
        xpool = ctx.enter_context(tc.tile_pool(name="x", bufs=4))
        hpool = ctx.enter_context(tc.tile_pool(name="h", bufs=3))
        bpool = ctx.enter_context(tc.tile_pool(name="b", bufs=1))
        ypool = ctx.enter_context(tc.tile_pool(name="y", bufs=3))
        ps1 = ctx.enter_context(tc.tile_pool(name="ps1", bufs=3, space="PSUM"))
        ps2 = ctx.enter_context(tc.tile_pool(name="ps2", bufs=3, space="PSUM"))

        w1_t, w2_t = {}, {}

        def issue_w1(e, eng_slots):
            t = wpool.tile([P, KD, 512], BF16, tag=f"w1_{e}", name=f"w1_{e}")
            nb = len(eng_slots)
            rows = P // nb
            for b, eng in enumerate(eng_slots):
                r0 = e * P + b * rows
                eng.dma_start(
                    t[b * rows:(b + 1) * rows, :, :],
                    w1_h[r0:r0 + rows, :].rearrange("p (kd f) -> p kd f", kd=KD),
                )
            w1_t[e] = t

        def issue_w2(e, eng_slots):
            t = wpool.tile([P, NFL, 1024], BF16, tag=f"w2_{e}", name=f"w2_{e}")
            nb = len(eng_slots)
            rows = P // nb
            for b, eng in enumerate(eng_slots):
                r0 = e * P + b * rows
                eng.dma_start(
                    t[b * rows:(b + 1) * rows, :, :],
                    w2_h[r0:r0 + rows, :].rearrange("p (fi d) -> p fi d", fi=NFL),
                )
            w2_t[e] = t

        def issue_xs(c, nb=4, eng=None):
            eng = eng or nc.sync
            csz = CH_PLAN[c]
            t = xpool.tile([P, KD, csz], BF16, tag="x", name=f"x{c}")
            rows = P // nb
            for b in range(nb):
                r0 = c * P + b * rows
                eng.dma_start(
                    t[b * rows:(b + 1) * rows, :, :],
                    xs_h[r0:r0 + rows, :KD * csz].rearrange(
                        "p (kd t) -> p kd t", kd=KD),
                )
            return t

        # ---- head: critical prefetches ----
        b1_raw = bpool.tile([P, E * NFL], F32)
        nc.gpsimd.dma_start(b1_raw[:], b1_h[:])
        x_t = {0: issue_xs(0, nb=8)}
        issue_w1(0, [nc.scalar] * 8)
        issue_w2(0, [nc.gpsimd] * 8)
        # Funnel b1 through an ACT-engine copy: downstream gelus then reach it
        # via same-engine program order instead of an extra semaphore wait.
        b1_t = bpool.tile([P, E * NFL], F32)
        nc.scalar.copy(b1_t[:], b1_raw[:])
        x_t[1] = issue_xs(1, nb=8)
        x_t[2] = issue_xs(2, nb=8)

        gelu = mybir.ActivationFunctionType.Gelu
        pend = None  # (c, csz, h_tile) awaiting GEMM2 (software pipeline)

        def do_g2(c, csz, h_t):
            y_t = ypool.tile([P, ND, csz], BF16, tag="y", name=f"y{c}")
            for dd in range(ND):
                pt2 = ps2.tile([P, csz], F32, tag="ps2", name="pt2")
                for (e, s0, s1) in segs[c]:
                    for fi in range(NFL):
                        nc.tensor.matmul(
                            pt2[:, s0:s1],
                            w2_t[e][:, fi, dd * P:(dd + 1) * P],
                            h_t[:, fi, s0:s1],
                            start=(fi == 0),
                            stop=(fi == NFL - 1),
                        )
                ceng = nc.vector if dd % 2 == 0 else nc.scalar
                if ceng is nc.vector:
                    ceng.tensor_copy(y_t[:, dd, :], pt2[:, :])
                else:
                    ceng.copy(y_t[:, dd, :], pt2[:, :])
            yeng = nc.sync if c >= NCH - 2 else nc.gpsimd
            for b in range(2):
                rows = P // 2
                r0 = c * P + b * rows
                yeng.dma_start(
                    y_h[r0:r0 + rows, :ND * csz].rearrange(
                        "p (dd t) -> p dd t", dd=ND),
                    y_t[b * rows:(b + 1) * rows, :, :],
                )

        for c in range(NCH):
            csz = CH_PLAN[c]
            if c + 3 < NCH:
                x_t[c + 3] = issue_xs(c + 3, nb=8 if c == 0 else 4)
            for e in due[c]:
                issue_w1(e, [nc.scalar] * 2 + [nc.gpsimd] * 2)
                issue_w2(e, [nc.sync] * 2 + [nc.gpsimd] * 2)
            h_t = hpool.tile([P, NFL, csz], BF16, tag="h", name=f"h{c}")
            for fi in range(NFL):
                pt = ps1.tile([P, csz], F32, tag="ps1", name="pt1")
                for (e, s0, s1) in segs[c]:
                    for k in range(KD):
                        nc.tensor.matmul(
                            pt[:, s0:s1],
                            w1_t[e][:, k, fi * P:(fi + 1) * P],
                            x_t[c][:, k, s0:s1],
                            start=(k == 0),
                            stop=(k == KD - 1),
                        )
                for (e, s0, s1) in segs[c]:
                    nc.scalar.activation(
                        h_t[:, fi, s0:s1], pt[:, s0:s1], gelu,
                        bias=b1_t[:, e * NFL + fi:e * NFL + fi + 1],
                    )
            if pend is not None:
                do_g2(*pend)
            pend = (c, csz, h_t)
            x_t.pop(c, None)
        do_g2(*pend)
    return _patch_to_json(nc)


def _route(xf: np.ndarray, gate_w: np.ndarray):
    """Top-2 gating identical to the reference (argmax ties -> lower index)."""
    N = xf.shape[0]
    logits = xf @ gate_w  # (N, E) f32
    rows = np.arange(N)
    i1 = logits.argmax(1)
    v1 = logits[rows, i1]
    masked = logits.copy()
    masked[rows, i1] = -np.inf
    i2 = masked.argmax(1)
    v2 = masked[rows, i2]
    # softmax over the two selected logits (v1 >= v2)
    e = np.exp((v2 - v1).astype(np.float32))
    wt1 = (1.0 / (1.0 + e)).astype(np.float32)
    wt2 = (e / (1.0 + e)).astype(np.float32)
    idx_e, wts_e = [], []
    for ex in range(E):
        s1 = np.nonzero(i1 == ex)[0]
        s2 = np.nonzero(i2 == ex)[0]
        idx_e.append(np.concatenate([s1, s2]))
        wts_e.append(np.concatenate([wt1[s1], wt2[s2]]).astype(np.float32))
    return idx_e, wts_e


def kernel(x, gate_w, w1, b1, w2, b2, _trace=False):
    B, T, D_ = x.shape
    N = B * T
    xf = np.ascontiguousarray(x.reshape(N, D_).astype(np.float32))
    idx_e, wts_e = _route(xf, gate_w.astype(np.float32))
    cnts = tuple(len(i) for i in idx_e)

    if cnts in _neff_cache:
        nc = _neff_cache[cnts]
    else:
        nc = _build_bass(cnts)
        _neff_cache[cnts] = nc

    order = np.concatenate(idx_e)
    xg = xf[order]  # (NPAIR, D)

    # xs[c*P+p, kd*csz+col] = xg[o0+col, kd*P+p]
    xs = np.zeros((NCH * P, KD * 512), np.float32)
    for c in range(NCH):
        o0, o1 = CH_OFF[c], CH_OFF[c + 1]
        csz = o1 - o0
        blk = xg[o0:o1].T.reshape(KD, P, csz).transpose(1, 0, 2)
        xs[c * P:(c + 1) * P, :KD * csz] = blk.reshape(P, KD * csz)
    xs = xs.astype(NP_BF16)

    in_maps = []
    for s in range(E):
        sl = slice(s * S_LOC, (s + 1) * S_LOC)
        w1x = (
            w1[:, :, sl].reshape(E, KD, P, S_LOC)
            .transpose(0, 2, 1, 3).reshape(E * P, KD * S_LOC)
        )
        w2x = (
            w2[:, sl, :].reshape(E, NFL, P, D)
            .transpose(0, 2, 1, 3).reshape(E * P, NFL * D)
        )
        b1x = (
            b1[:, sl].reshape(E, NFL, P)
            .transpose(2, 0, 1).reshape(P, E * NFL)
        )
        in_maps.append({
            "xs": xs,
            "w1": np.ascontiguousarray(w1x).astype(NP_BF16),
            "w2": np.ascontiguousarray(w2x).astype(NP_BF16),
            "b1": np.ascontiguousarray(b1x).astype(np.float32),
        })

    res = run_bass_kernel_spmd(nc, in_maps, core_ids=list(range(E)), trace=_trace)
    if _trace:
        print(f"HW exec time: {res.exec_time_ns} ns")

    ysum = np.zeros((NCH * P, ND * 512), np.float32)
    for s in range(E):
        ysum += res.results[s]["y"].astype(np.float32)

    # unpack: y_part[o0+col, dd*P+p] = ysum[c*P+p, dd*csz+col]
    yp = np.empty((NPAIR, D), np.float32)
    for c in range(NCH):
        o0, o1 = CH_OFF[c], CH_OFF[c + 1]
        csz = o1 - o0
        blk = ysum[c * P:(c + 1) * P, :ND * csz].reshape(P, ND, csz)
        yp[o0:o1] = blk.transpose(2, 1, 0).reshape(csz, D)

    out = np.zeros((N, D), np.float32)
    off = 0
    for ex in range(E):
        cnt = cnts[ex]
        if not cnt:
            continue
        yv = yp[off:off + cnt] + b2[ex][None, :].astype(np.float32)
        out[idx_e[ex]] += wts_e[ex][:, None] * yv
        off += cnt
    return out.reshape(B, T, D_)


# revision 11
# speedup vs baseline: 1.0573x; 1.0573x over previous
# MoE kernel for Trainium2 (8 NeuronCores, dff-sharded expert MLP).
#
# Strategy:
#  - Host: gate logits = x @ gate_w, top-2 + softmax, gather tokens per expert
#    into one expert-sorted pair stream of exactly N*TOP_K = 8192 tokens.
#  - Device (core s = dff slice s): every core processes ALL 8192 routed
#    pairs over a 512-wide slice of d_ff: h = gelu(x @ w1[:, slice]);
#    y_partial = h @ w2[slice, :]. Perfectly load-balanced regardless of
#    routing (the expert-parallel alternative pads every core to the max
#    expert load). Token stream is cut into fixed chunks; expert boundaries
#    fall inside chunks as column sub-ranges of the same PSUM tile.
#  - Host: sum the 8 bf16 partials, add b2, scatter-add wts * y back.
import math
from contextlib import ExitStack

import ml_dtypes
import numpy as np

import concourse.bass as bass
import concourse.mybir as mybir
import concourse.tile as tile
from concourse.bass_utils import run_bass_kernel_spmd

D = 1024
DFF = 4096
E = 8
TOP_K = 2
P = 128
KD = D // P        # 8 contraction tiles for GEMM1
S_LOC = DFF // 8   # 512 dff columns per core
NFL = S_LOC // P   # 4 local dff tiles (GEMM1 out / GEMM2 contraction)
ND = D // P        # 8 GEMM2 out tiles
NPAIR = 4096 * TOP_K
CH_PLAN = [128, 384] + [512] * 14 + [384, 128]
assert sum(CH_PLAN) == NPAIR
NCH = len(CH_PLAN)
CH_OFF = [0]
for _s in CH_PLAN:
    CH_OFF.append(CH_OFF[-1] + _s)

BF16 = mybir.dt.bfloat16
F32 = mybir.dt.float32
NP_BF16 = np.dtype(ml_dtypes.bfloat16)

_neff_cache = {}


def _split_multiwait_json(bir_bytes: bytes) -> bytes:
    """The walrus build in this container rejects instructions carrying more
    than one sync wait (or update). Split extras onto adjacent single-wait
    EventSemaphore carriers on the same engine: program order on the engine
    preserves the semantics exactly."""
    import json as _json

    bir = _json.loads(bir_bytes)
    for fn in bir["functions"]:
        for blk in fn["blocks"]:
            insts = blk.get("instructions", [])
            out = []
            for inst in insts:
                si = inst.get("sync_info")
                if si:
                    waits = si.get("on_wait") or []
                    if len(waits) > 1:
                        for i, w in enumerate(waits[:-1]):
                            out.append({
                                "debug": inst.get("debug", 0),
                                "engine": inst["engine"],
                                "ins": [],
                                "name": f"{inst['name']}_w{i}",
                                "opcode": "EventSemaphore",
                                "outs": [],
                                "sync_info": {"on_update": [], "on_wait": [w]},
                            })
                        si["on_wait"] = [waits[-1]]
                out.append(inst)
                if si:
                    ups = si.get("on_update") or []
                    if len(ups) > 1:
                        for i, u in enumerate(ups[1:]):
                            out.append({
                                "debug": inst.get("debug", 0),
                                "engine": inst["engine"],
                                "ins": [],
                                "name": f"{inst['name']}_u{i}",
                                "opcode": "EventSemaphore",
                                "outs": [],
                                "sync_info": {"on_update": [u], "on_wait": []},
                            })
                        si["on_update"] = [ups[0]]
            blk["instructions"] = out
    return _json.dumps(bir).encode()


def _patch_to_json(nc: bass.Bass) -> bass.Bass:
    orig = nc.to_json_bytes
    nc.to_json_bytes = lambda: _split_multiwait_json(orig())
    return nc


def _segments(cnts):
    """Per chunk: list of (expert, col0, col1) in chunk-local columns."""
    st = [0]
    for c in cnts:
        st.append(st[-1] + c)
    segs = []
    for ci in range(NCH):
        o0, o1 = CH_OFF[ci], CH_OFF[ci + 1]
        lst = []
        for e in range(E):
            a, b = max(o0, st[e]), min(o1, st[e + 1])
            if a < b:
                lst.append((e, a - o0, b - o0))
        segs.append(lst)
    return segs


def _build_bass(cnts) -> bass.Bass:
    """One dff-slice of the MoE MLP; identical program on all 8 cores.

    DRAM layouts (rows padded to fixed 8KB width; host packs accordingly):
      xs : [NCH*P, KD*512] bf16; rows c*P+p hold [kd, csz] = xg[o0+col, kd*P+p]
      w1 : [E*P, KD*512] bf16; rows e*P+p hold [kd, f] = w1[e][kd*P+p, slice f]
      w2 : [E*P, NFL*1024] bf16; rows e*P+p hold [fi, dm] = w2[e][slice fi*P+p, dm]
      b1 : [P, E*NFL] f32; [p, e*NFL+fi] = b1[e][slice fi*P+p]
      y  : [NCH*P, ND*512] bf16; rows c*P+p hold [dd, csz] = y_part[o0+col, dd*P+p]
    """
    nc = bass.Bass()
    segs = _segments(cnts)
    xs_h = nc.dram_tensor("xs", [NCH * P, KD * 512], BF16, kind="ExternalInput")
    w1_h = nc.dram_tensor("w1", [E * P, KD * 512], BF16, kind="ExternalInput")
    w2_h = nc.dram_tensor("w2", [E * P, NFL * 1024], BF16, kind="ExternalInput")
    b1_h = nc.dram_tensor("b1", [P, E * NFL], F32, kind="ExternalInput")
    y_h = nc.dram_tensor("y", [NCH * P, ND * 512], BF16, kind="ExternalOutput")

    # first chunk index where each expert appears -> weight prefetch schedule
    first_chunk = {}
    for ci in range(NCH):
        for (e, _, _) in segs[ci]:
            if e not in first_chunk:
                first_chunk[e] = ci
    due = {ci: [] for ci in range(NCH)}
    for e in range(E):
        if e == 0 or e not in first_chunk:
            continue
        due[max(0, first_chunk[e] - 3)].append(e)

    with ExitStack() as ctx:
        tc = ctx.enter_context(tile.TileContext(nc))
        wpool = ctx.enter_context(tc.tile_pool(name="w", bufs=1))
        xpool = ctx.enter_context(tc.tile_pool(name="x", bufs=4))
        hpool = ctx.enter_context(tc.tile_pool(name="h", bufs=3))
        bpool = ctx.enter_context(tc.tile_pool(name="b", bufs=1))
        ypool = ctx.enter_context(tc.tile_pool(name="y", bufs=3))
        ps1 = ctx.enter_context(tc.tile_pool(name="ps1", bufs=3, space="PSUM"))
        ps2 = ctx.enter_context(tc.tile_pool(name="ps2", bufs=3, space="PSUM"))

        w1_t, w2_t = {}, {}

        def issue_w1(e, eng_slots):
            t = wpool.tile([P, KD, 512], BF16, tag=f"w1_{e}", name=f"w1_{e}")
            nb = len(eng_slots)
            rows = P // nb
            for b, eng in enumerate(eng_slots):
                r0 = e * P + b * rows
                eng.dma_start(
                    t[b * rows:(b + 1) * rows, :, :],
                    w1_h[r0:r0 + rows, :].rearrange("p (kd f) -> p kd f", kd=KD),
                )
            w1_t[e] = t

        def issue_w2(e, eng_slots):
            t = wpool.tile([P, NFL, 1024], BF16, tag=f"w2_{e}", name=f"w2_{e}")
            nb = len(eng_slots)
            rows = P // nb
            for b, eng in enumerate(eng_slots):
                r0 = e * P + b * rows
                eng.dma_start(
                    t[b * rows:(b + 1) * rows, :, :],
                    w2_h[r0:r0 + rows, :].rearrange("p (fi d) -> p fi d", fi=NFL),
                )
            w2_t[e] = t

        def issue_xs(c, nb=4, eng=None):
            eng = eng or nc.sync
            csz = CH_PLAN[c]
            t = xpool.tile([P, KD, csz], BF16, tag="x", name=f"x{c}")
            rows = P // nb
            for b in range(nb):
                r0 = c * P + b * rows
                eng.dma_start(
                    t[b * rows:(b + 1) * rows, :, :],
                    xs_h[r0:r0 + rows, :KD * csz].rearrange(
                        "p (kd t) -> p kd t", kd=KD),
                )
            return t

        # ---- head: critical prefetches ----
        b1_raw = bpool.tile([P, E * NFL], F32)
        nc.gpsimd.dma_start(b1_raw[:], b1_h[:])
        x_t = {0: issue_xs(0, nb=4)}
        issue_w1(0, [nc.scalar] * 4 + [nc.gpsimd] * 4)
        # Funnel b1 through an ACT-engine copy: downstream gelus then reach it
        # via same-engine program order instead of an extra semaphore wait.
        b1_t = bpool.tile([P, E * NFL], F32)
        nc.scalar.copy(b1_t[:], b1_raw[:])
        x_t[1] = issue_xs(1, nb=4)
        x_t[2] = issue_xs(2, nb=4)
        issue_w2(0, [nc.sync] * 4 + [nc.gpsimd] * 4)

        gelu = mybir.ActivationFunctionType.Gelu
        pend = None  # (c, csz, h_tile) awaiting GEMM2 (software pipeline)

        def do_g2(c, csz, h_t):
            y_t = ypool.tile([P, ND, csz], BF16, tag="y", name=f"y{c}")
            for dd in range(ND):
                pt2 = ps2.tile([P, csz], F32, tag="ps2", name="pt2")
                for (e, s0, s1) in segs[c]:
                    for fi in range(NFL):
                        nc.tensor.matmul(
                            pt2[:, s0:s1],
                            w2_t[e][:, fi, dd * P:(dd + 1) * P],
                            h_t[:, fi, s0:s1],
                            start=(fi == 0),
                            stop=(fi == NFL - 1),
                        )
                if c >= NCH - 2 and dd % 2 == 1:
                    nc.scalar.copy(y_t[:, dd, :], pt2[:, :])
                else:
                    nc.vector.tensor_copy(y_t[:, dd, :], pt2[:, :])
            yeng = nc.sync if c >= NCH - 2 else nc.gpsimd
            for b in range(2):
                rows = P // 2
                r0 = c * P + b * rows
                yeng.dma_start(
                    y_h[r0:r0 + rows, :ND * csz].rearrange(
                        "p (dd t) -> p dd t", dd=ND),
                    y_t[b * rows:(b + 1) * rows, :, :],
                )

        for c in range(NCH):
            csz = CH_PLAN[c]
            if c + 3 < NCH:
                x_t[c + 3] = issue_xs(c + 3, nb=4)
            for e in due[c]:
                issue_w1(e, [nc.scalar] * 2 + [nc.gpsimd] * 2)
                issue_w2(e, [nc.sync] * 2 + [nc.gpsimd] * 2)
            h_t = hpool.tile([P, NFL, csz], BF16, tag="h", name=f"h{c}")
            for fi in range(NFL):
                pt = ps1.tile([P, csz], F32, tag="ps1", name="pt1")
                for (e, s0, s1) in segs[c]:
                    for k in range(KD):
                        nc.tensor.matmul(
                            pt[:, s0:s1],
                            w1_t[e][:, k, fi * P:(fi + 1) * P],
                            x_t[c][:, k, s0:s1],
                            start=(k == 0),
                            stop=(k == KD - 1),
                        )
                for (e, s0, s1) in segs[c]:
                    nc.scalar.activation(
                        h_t[:, fi, s0:s1], pt[:, s0:s1], gelu,
                        bias=b1_t[:, e * NFL + fi:e * NFL + fi + 1],
                    )
            if pend is not None:
                do_g2(*pend)
            pend = (c, csz, h_t)
            x_t.pop(c, None)
        do_g2(*pend)
    return _patch_to_json(nc)


def _route(xf: np.ndarray, gate_w: np.ndarray):
    """Top-2 gating identical to the reference (argmax ties -> lower index)."""
    N = xf.shape[0]
    logits = xf @ gate_w  # (N, E) f32
    rows = np.arange(N)
    i1 = logits.argmax(1)
    v1 = logits[rows, i1]
    masked = logits.copy()
    masked[rows, i1] = -np.inf
    i2 = masked.argmax(1)
    v2 = masked[rows, i2]
    # softmax over the two selected logits (v1 >= v2)
    e = np.exp((v2 - v1).astype(np.float32))
    wt1 = (1.0 / (1.0 + e)).astype(np.float32)
    wt2 = (e / (1.0 + e)).astype(np.float32)
    idx_e, wts_e = [], []
    for ex in range(E):
        s1 = np.nonzero(i1 == ex)[0]
        s2 = np.nonzero(i2 == ex)[0]
        idx_e.append(np.concatenate([s1, s2]))
        wts_e.append(np.concatenate([wt1[s1], wt2[s2]]).astype(np.float32))
    return idx_e, wts_e


def kernel(x, gate_w, w1, b1, w2, b2, _trace=False):
    B, T, D_ = x.shape
    N = B * T
    xf = np.ascontiguousarray(x.reshape(N, D_).astype(np.float32))
    idx_e, wts_e = _route(xf, gate_w.astype(np.float32))
    cnts = tuple(len(i) for i in idx_e)

    if cnts in _neff_cache:
        nc = _neff_cache[cnts]
    else:
        nc = _build_bass(cnts)
        _neff_cache[cnts] = nc

    order = np.concatenate(idx_e)
    xg = xf[order]  # (NPAIR, D)

    # xs[c*P+p, kd*csz+col] = xg[o0+col, kd*P+p]
    xs = np.zeros((NCH * P, KD * 512), np.float32)
    for c in range(NCH):
        o0, o1 = CH_OFF[c], CH_OFF[c + 1]
        csz = o1 - o0
        blk = xg[o0:o1].T.reshape(KD, P, csz).transpose(1, 0, 2)
        xs[c * P:(c + 1) * P, :KD * csz] = blk.reshape(P, KD * csz)
    xs = xs.astype(NP_BF16)

    in_maps = []
    for s in range(E):
        sl = slice(s * S_LOC, (s + 1) * S_LOC)
        w1x = (
            w1[:, :, sl].reshape(E, KD, P, S_LOC)
            .transpose(0, 2, 1, 3).reshape(E * P, KD * S_LOC)
        )
        w2x = (
            w2[:, sl, :].reshape(E, NFL, P, D)
            .transpose(0, 2, 1, 3).reshape(E * P, NFL * D)
        )
        b1x = (
            b1[:, sl].reshape(E, NFL, P)
            .transpose(2, 0, 1).reshape(P, E * NFL)
        )
        in_maps.append({
            "xs": xs,
            "w1": np.ascontiguousarray(w1x).astype(NP_BF16),
            "w2": np.ascontiguousarray(w2x).astype(NP_BF16),
            "b1": np.ascontiguousarray(b1x).astype(np.float32),
        })

    res = run_bass_kernel_spmd(nc, in_maps, core_ids=list(range(E)), trace=_trace)
    if _trace:
        print(f"HW exec time: {res.exec_time_ns} ns")

    ysum = np.zeros((NCH * P, ND * 512), np.float32)
    for s in range(E):
        ysum += res.results[s]["y"].astype(np.float32)

    # unpack: y_part[o0+col, dd*P+p] = ysum[c*P+p, dd*csz+col]
    yp = np.empty((NPAIR, D), np.float32)
    for c in range(NCH):
        o0, o1 = CH_OFF[c], CH_OFF[c + 1]
        csz = o1 - o0
        blk = ysum[c * P:(c + 1) * P, :ND * csz].reshape(P, ND, csz)
        yp[o0:o1] = blk.transpose(2, 1, 0).reshape(csz, D)

    out = np.zeros((N, D), np.float32)
    off = 0
    for ex in range(E):
        cnt = cnts[ex]
        if not cnt:
            continue
        yv = yp[off:off + cnt] + b2[ex][None, :].astype(np.float32)
        out[idx_e[ex]] += wts_e[ex][:, None] * yv
        off += cnt
    return out.reshape(B, T, D_)


# revision 20
# speedup vs baseline: 1.0636x; 1.0060x over previous
# MoE kernel for Trainium2 (8 NeuronCores, dff-sharded expert MLP).
#
# Strategy:
#  - Host: gate logits = x @ gate_w, top-2 + softmax, gather tokens per expert
#    into one expert-sorted pair stream of exactly N*TOP_K = 8192 tokens.
#  - Device (core s = dff slice s): every core processes ALL 8192 routed
#    pairs over a 512-wide slice of d_ff: h = gelu(x @ w1[:, slice]);
#    y_partial = h @ w2[slice, :]. Perfectly load-balanced regardless of
#    routing (the expert-parallel alternative pads every core to the max
#    expert load). Token stream is cut into fixed chunks; expert boundaries
#    fall inside chunks as column sub-ranges of the same PSUM tile.
#  - Host: sum the 8 bf16 partials, add b2, scatter-add wts * y back.
import math
from contextlib import ExitStack

import ml_dtypes
import numpy as np

import concourse.bass as bass
import concourse.mybir as mybir
import concourse.tile as tile
from concourse.bass_utils import run_bass_kernel_spmd

D = 1024
DFF = 4096
E = 8
TOP_K = 2
P = 128
KD = D // P        # 8 contraction tiles for GEMM1
S_LOC = DFF // 8   # 512 dff columns per core
NFL = S_LOC // P   # 4 local dff tiles (GEMM1 out / GEMM2 contraction)
ND = D // P        # 8 GEMM2 out tiles
NPAIR = 4096 * TOP_K
CH_PLAN = [128, 384] + [512] * 14 + [320, 128, 64]
assert sum(CH_PLAN) == NPAIR
NCH = len(CH_PLAN)
CH_OFF = [0]
for _s in CH_PLAN:
    CH_OFF.append(CH_OFF[-1] + _s)

BF16 = mybir.dt.bfloat16
F32 = mybir.dt.float32
NP_BF16 = np.dtype(ml_dtypes.bfloat16)

_neff_cache = {}


def _split_multiwait_json(bir_bytes: bytes) -> bytes:
    """The walrus build in this container rejects instructions carrying more
    than one sync wait (or update). Split extras onto adjacent single-wait
    EventSemaphore carriers on the same engine: program order on the engine
    preserves the semantics exactly."""
    import json as _json

    bir = _json.loads(bir_bytes)
    for fn in bir["functions"]:
        for blk in fn["blocks"]:
            insts = blk.get("instructions", [])
            out = []
            for inst in insts:
                si = inst.get("sync_info")
                if si:
                    waits = si.get("on_wait") or []
                    if len(waits) > 1:
                        for i, w in enumerate(waits[:-1]):
                            out.append({
                                "debug": inst.get("debug", 0),
                                "engine": inst["engine"],
                                "ins": [],
                                "name": f"{inst['name']}_w{i}",
                                "opcode": "EventSemaphore",
                                "outs": [],
                                "sync_info": {"on_update": [], "on_wait": [w]},
                            })
                        si["on_wait"] = [waits[-1]]
                out.append(inst)
                if si:
                    ups = si.get("on_update") or []
                    if len(ups) > 1:
                        for i, u in enumerate(ups[1:]):
                            out.append({
                                "debug": inst.get("debug", 0),
                                "engine": inst["engine"],
                                "ins": [],
                                "name": f"{inst['name']}_u{i}",
                                "opcode": "EventSemaphore",
                                "outs": [],
                                "sync_info": {"on_update": [u], "on_wait": []},
                            })
                        si["on_update"] = [ups[0]]
            blk["instructions"] = out
    return _json.dumps(bir).encode()


def _patch_to_json(nc: bass.Bass) -> bass.Bass:
    orig = nc.to_json_bytes
    nc.to_json_bytes = lambda: _split_multiwait_json(orig())
    return nc


def _segments(cnts):
    """Per chunk: list of (expert, col0, col1) in chunk-local columns."""
    st = [0]
    for c in cnts:
        st.append(st[-1] + c)
    segs = []
    for ci in range(NCH):
        o0, o1 = CH_OFF[ci], CH_OFF[ci + 1]
        lst = []
        for e in range(E):
            a, b = max(o0, st[e]), min(o1, st[e + 1])
            if a < b:
                lst.append((e, a - o0, b - o0))
        segs.append(lst)
    return segs


def _build_bass(cnts) -> bass.Bass:
    """One dff-slice of the MoE MLP; identical program on all 8 cores.

    DRAM layouts (rows padded to fixed 8KB width; host packs accordingly):
      xs : [NCH*P, KD*512] bf16; rows c*P+p hold [kd, csz] = xg[o0+col, kd*P+p]
      w1 : [E*P, KD*512] bf16; rows e*P+p hold [kd, f] = w1[e][kd*P+p, slice f]
      w2 : [E*P, NFL*1024] bf16; rows e*P+p hold [fi, dm] = w2[e][slice fi*P+p, dm]
      b1 : [P, E*NFL] f32; [p, e*NFL+fi] = b1[e][slice fi*P+p]
      y  : [NCH*P, ND*512] bf16; rows c*P+p hold [dd, csz] = y_part[o0+col, dd*P+p]
    """
    nc = bass.Bass()
    segs = _segments(cnts)
    xs_h = nc.dram_tensor("xs", [NCH * P, KD * 512], BF16, kind="ExternalInput")
    # fi-major rows (2KB): row e*S_LOC+fi*P+p holds [k, m] = w1[e][k*P+p, fi*P+m]
    w1_h = nc.dram_tensor("w1", [E * S_LOC, KD * P], BF16, kind="ExternalInput")
    w2_h = nc.dram_tensor("w2", [E * P, NFL * 1024], BF16, kind="ExternalInput")
    b1_h = nc.dram_tensor("b1", [P, E * NFL], F32, kind="ExternalInput")
    y_h = nc.dram_tensor("y", [NCH * P, ND * 512], BF16, kind="ExternalOutput")

    # first chunk index where each expert appears -> weight prefetch schedule
    first_chunk = {}
    for ci in range(NCH):
        for (e, _, _) in segs[ci]:
            if e not in first_chunk:
                first_chunk[e] = ci
    due = {ci: [] for ci in range(NCH)}
    for e in range(E):
        if e == 0 or e not in first_chunk:
            continue
        due[max(0, first_chunk[e] - 3)].append(e)

    with ExitStack() as ctx:
        tc = ctx.enter_context(tile.TileContext(nc))
        wpool = ctx.enter_context(tc.tile_pool(name="w", bufs=1))
        xpool = ctx.enter_context(tc.tile_pool(name="x", bufs=3))
        hpool = ctx.enter_context(tc.tile_pool(name="h", bufs=3))
        bpool = ctx.enter_context(tc.tile_pool(name="b", bufs=1))
        ypool = ctx.enter_context(tc.tile_pool(name="y", bufs=3))
        ps1 = ctx.enter_context(tc.tile_pool(name="ps1", bufs=3, space="PSUM"))
        ps2 = ctx.enter_context(tc.tile_pool(name="ps2", bufs=3, space="PSUM"))

        w1_t, w2_t = {}, {}

        def issue_w1(e, eng, fi_bands):
            # fi-sliced delivery: the first matmul of an expert only needs
            # fi=0, so a fine split of fi0 lets GEMM1 start early.
            t = wpool.tile([P, NFL, KD, P], BF16, tag=f"w1_{e}", name=f"w1_{e}")
            for fi in range(NFL):
                nb = fi_bands[fi]
                rows = P // nb
                for b in range(nb):
                    r0 = e * S_LOC + fi * P + b * rows
                    eng.dma_start(
                        t[b * rows:(b + 1) * rows, fi, :, :],
                        w1_h[r0:r0 + rows, :].rearrange(
                            "p (k m) -> p k m", k=KD),
                    )
            w1_t[e] = t

        def issue_w2(e, eng, nb=4):
            t = wpool.tile([P, NFL, 1024], BF16, tag=f"w2_{e}", name=f"w2_{e}")
            rows = P // nb
            for b in range(nb):
                r0 = e * P + b * rows
                eng.dma_start(
                    t[b * rows:(b + 1) * rows, :, :],
                    w2_h[r0:r0 + rows, :].rearrange("p (fi d) -> p fi d", fi=NFL),
                )
            w2_t[e] = t

        def issue_xs(c, nb=4, eng=None):
            eng = eng or nc.sync
            csz = CH_PLAN[c]
            t = xpool.tile([P, KD, csz], BF16, tag="x", name=f"x{c}")
            rows = P // nb
            for b in range(nb):
                r0 = c * P + b * rows
                eng.dma_start(
                    t[b * rows:(b + 1) * rows, :, :],
                    xs_h[r0:r0 + rows, :KD * csz].rearrange(
                        "p (kd t) -> p kd t", kd=KD),
                )
            return t

        # ---- head ----
        # All head-critical transfers go on gpsimd (deep queue, never
        # stalls); the b1 shims also shift the ring round-robin so later
        # wrapped issues queue FIFO behind the critical bands.
        b1_raw = bpool.tile([P, E * NFL], F32)
        for b in range(4):
            nc.gpsimd.dma_start(b1_raw[:, b * 8:(b + 1) * 8],
                                b1_h[:, b * 8:(b + 1) * 8])
        x_t = {0: issue_xs(0, nb=4, eng=nc.gpsimd)}
        issue_w1(0, nc.gpsimd, [4, 2, 2, 2])
        x_t[1] = issue_xs(1, nb=4, eng=nc.gpsimd)
        x_t[2] = issue_xs(2, nb=4, eng=nc.gpsimd)
        issue_w2(0, nc.gpsimd, nb=8)
        # Funnel b1 through an ACT-engine copy: downstream gelus then reach it
        # via same-engine program order instead of an extra semaphore wait.
        b1_t = bpool.tile([P, E * NFL], F32)
        nc.scalar.copy(b1_t[:], b1_raw[:])

        gelu = mybir.ActivationFunctionType.Gelu
        pend = None  # (c, csz, h_tile) awaiting GEMM2 (software pipeline)

        def do_g2(c, csz, h_t):
            y_t = ypool.tile([P, ND, csz], BF16, tag="y", name=f"y{c}")
            for dd in range(ND):
                pt2 = ps2.tile([P, csz], F32, tag="ps2", name="pt2")
                for (e, s0, s1) in segs[c]:
                    for fi in range(NFL):
                        nc.tensor.matmul(
                            pt2[:, s0:s1],
                            w2_t[e][:, fi, dd * P:(dd + 1) * P],
                            h_t[:, fi, s0:s1],
                            start=(fi == 0),
                            stop=(fi == NFL - 1),
                        )
                if c >= NCH - 3 and dd % 2 == 1:
                    nc.scalar.copy(y_t[:, dd, :], pt2[:, :])
                else:
                    nc.vector.tensor_copy(y_t[:, dd, :], pt2[:, :])
            yeng = nc.sync if c >= NCH - 3 else nc.gpsimd
            nb = 4 if c >= NCH - 3 else 2
            rows = P // nb
            for b in range(nb):
                r0 = c * P + b * rows
                yeng.dma_start(
                    y_h[r0:r0 + rows, :ND * csz].rearrange(
                        "p (dd t) -> p dd t", dd=ND),
                    y_t[b * rows:(b + 1) * rows, :, :],
                )

        for c in range(NCH):
            csz = CH_PLAN[c]
            for e in due[c]:
                issue_w1(e, nc.gpsimd, [2, 2, 2, 2])
                issue_w2(e, nc.sync, nb=4)
            if c + 3 < NCH:
                x_t[c + 3] = issue_xs(c + 3, nb=4, eng=nc.sync)
            h_t = hpool.tile([P, NFL, csz], BF16, tag="h", name=f"h{c}")
            for fi in range(NFL):
                pt = ps1.tile([P, csz], F32, tag="ps1", name="pt1")
                for (e, s0, s1) in segs[c]:
                    for k in range(KD):
                        nc.tensor.matmul(
                            pt[:, s0:s1],
                            w1_t[e][:, fi, k, :],
                            x_t[c][:, k, s0:s1],
                            start=(k == 0),
                            stop=(k == KD - 1),
                        )
                for (e, s0, s1) in segs[c]:
                    nc.scalar.activation(
                        h_t[:, fi, s0:s1], pt[:, s0:s1], gelu,
                        bias=b1_t[:, e * NFL + fi:e * NFL + fi + 1],
                    )
            if pend is not None:
                do_g2(*pend)
            pend = (c, csz, h_t)
            x_t.pop(c, None)
        do_g2(*pend)
    return _patch_to_json(nc)


def _route(xf: np.ndarray, gate_w: np.ndarray):
    """Top-2 gating identical to the reference (argmax ties -> lower index)."""
    N = xf.shape[0]
    logits = xf @ gate_w  # (N, E) f32
    rows = np.arange(N)
    i1 = logits.argmax(1)
    v1 = logits[rows, i1]
    masked = logits.copy()
    masked[rows, i1] = -np.inf
    i2 = masked.argmax(1)
    v2 = masked[rows, i2]
    # softmax over the two selected logits (v1 >= v2)
    e = np.exp((v2 - v1).astype(np.float32))
    wt1 = (1.0 / (1.0 + e)).astype(np.float32)
    wt2 = (e / (1.0 + e)).astype(np.float32)
    idx_e, wts_e = [], []
    for ex in range(E):
        s1 = np.nonzero(i1 == ex)[0]
        s2 = np.nonzero(i2 == ex)[0]
        idx_e.append(np.concatenate([s1, s2]))
        wts_e.append(np.concatenate([wt1[s1], wt2[s2]]).astype(np.float32))
    return idx_e, wts_e


def kernel(x, gate_w, w1, b1, w2, b2, _trace=False):
    B, T, D_ = x.shape
    N = B * T
    xf = np.ascontiguousarray(x.reshape(N, D_).astype(np.float32))
    idx_e, wts_e = _route(xf, gate_w.astype(np.float32))
    cnts = tuple(len(i) for i in idx_e)

    if cnts in _neff_cache:
        nc = _neff_cache[cnts]
    else:
        nc = _build_bass(cnts)
        _neff_cache[cnts] = nc

    order = np.concatenate(idx_e)
    xg = xf[order]  # (NPAIR, D)

    # xs[c*P+p, kd*csz+col] = xg[o0+col, kd*P+p]
    xs = np.zeros((NCH * P, KD * 512), np.float32)
    for c in range(NCH):
        o0, o1 = CH_OFF[c], CH_OFF[c + 1]
        csz = o1 - o0
        blk = xg[o0:o1].T.reshape(KD, P, csz).transpose(1, 0, 2)
        xs[c * P:(c + 1) * P, :KD * csz] = blk.reshape(P, KD * csz)
    xs = xs.astype(NP_BF16)

    in_maps = []
    for s in range(E):
        sl = slice(s * S_LOC, (s + 1) * S_LOC)
        # fi-major: row e*S_LOC+fi*P+p holds [k, m]
        w1x = (
            w1[:, :, sl].reshape(E, KD, P, NFL, P)
            .transpose(0, 3, 2, 1, 4).reshape(E * S_LOC, KD * P)
        )
        w2x = (
            w2[:, sl, :].reshape(E, NFL, P, D)
            .transpose(0, 2, 1, 3).reshape(E * P, NFL * D)
        )
        b1x = (
            b1[:, sl].reshape(E, NFL, P)
            .transpose(2, 0, 1).reshape(P, E * NFL)
        )
        in_maps.append({
            "xs": xs,
            "w1": np.ascontiguousarray(w1x).astype(NP_BF16),
            "w2": np.ascontiguousarray(w2x).astype(NP_BF16),
            "b1": np.ascontiguousarray(b1x).astype(np.float32),
        })

    res = run_bass_kernel_spmd(nc, in_maps, core_ids=list(range(E)), trace=_trace)
    if _trace:
        print(f"HW exec time: {res.exec_time_ns} ns")

    ysum = np.zeros((NCH * P, ND * 512), np.float32)
    for s in range(E):
        ysum += res.results[s]["y"].astype(np.float32)

    # unpack: y_part[o0+col, dd*P+p] = ysum[c*P+p, dd*csz+col]
    yp = np.empty((NPAIR, D), np.float32)
    for c in range(NCH):
        o0, o1 = CH_OFF[c], CH_OFF[c + 1]
        csz = o1 - o0
        blk = ysum[c * P:(c + 1) * P, :ND * csz].reshape(P, ND, csz)
        yp[o0:o1] = blk.transpose(2, 1, 0).reshape(csz, D)

    out = np.zeros((N, D), np.float32)
    off = 0
    for ex in range(E):
        cnt = cnts[ex]
        if not cnt:
            continue
        yv = yp[off:off + cnt] + b2[ex][None, :].astype(np.float32)
        out[idx_e[ex]] += wts_e[ex][:, None] * yv
        off += cnt
    return out.reshape(B, T, D_)
